# revision 13
# baseline (speedup 1.0000x reference)
"""Trainium2 Bass kernel for ColoredMLP (4-expert MoE over 500k edges).

Strategy (expert-parallel over colors, 2 cores per color):
  - Host groups edges by color (stable), pads each color segment to 126976,
    and assigns 2 cores per color.  Every core then runs an IDENTICAL dense
    single-expert MLP on 63488 edges with its own weight slice bound as
    inputs, so the device kernel is completely regular: no routing on
    device, no wasted all-expert compute.
  - Data layout: host ships x pre-transposed as [g, e] and interleaved so
    each DMA spans all 128 SBUF partitions: within a chunk of size S,
    x_in[t*64+g, e] = xT[g, t*(S/2) + e].  L1 uses zero-padded
    block-diagonal W1 slices (w1a = [W1;0], w1b = [0;W1]) so the matmul
    rhs is always a full-partition tile at base partition 0.
  - shifted_softplus(x) = softplus(x) - log2 is folded into the second
    layer's bias on host: b2_eff = b2 - log2 * W2.sum(0).  The device
    computes softplus as Ln(Exp(h + b1) + 1) — two ACT passes from the
    same activation-table set (this act_info has no native Softplus),
    batched wide to amortize ACT per-instruction overhead.  b1 rides the
    Exp pass as a per-partition ACT bias; b2_eff is a per-partition DVE
    scalar add fused into the PSUM->SBUF copy of y.
  - Matmuls run as float32r (full PE rate at N=512); PSUM accumulates fp32.
  - Input DMAs ride SP HWDGE queues, output DMAs ride the Activation
    HWDGE queues so output bursts never head-of-line-block input
    prefetch dispatch.

The kernel returns out^T tiles [128, 63488] per core; the host scatters
them back through the color permutation.
"""

import os
import sys

import numpy as np

if "/opt/trn_rl_repo" not in sys.path:
    sys.path.insert(0, "/opt/trn_rl_repo")

import bass_rust as _bass_rust
import concourse.bacc as bacc
import concourse.mybir as mybir
from concourse.hw_specs import get_activation_tables
from concourse.tile import TileContext
from concourse.bass_utils import run_bass_kernel_spmd


class _Bacc(bacc.Bacc):
    """Bacc that pins activation-table selection to the single set holding
    both Exp and Ln.  The default per-function choice alternates between
    `exp_and_others` and `natural_log`, inserting a ~1.3us ACT_TABLE_LOAD
    before every activation (82us of pure table thrash per core here)."""

    def insert_act_table_loads(self):
        has_activation = any(
            isinstance(i, mybir.InstActivation)
            for b in self.main_func.blocks
            for i in b.instructions
        )
        if not has_activation:
            return
        both = {
            mybir.ActivationFunctionType.Exp,
            mybir.ActivationFunctionType.Ln,
        }
        tables = []
        seen = False
        for k, fns in get_activation_tables(self.m.arch).items():
            if k == "natural_log_exp_and_others":
                seen = True
                assert both <= set(fns)
            else:
                fns = set(fns) - both
            tables.append((k, fns))
        assert seen, "natural_log_exp_and_others table set missing"
        _bass_rust.insert_act_table_loads(self, tables)


E, G, F, C = 500000, 64, 128, 4
N_CORES = 8
CHUNKS = [4096] * 15 + [2048]   # edges per DMA chunk
E_CORE = sum(CHUNKS)            # 63488 edges per core (fixed compile shape)
SEG = 2 * E_CORE                # 126976 padded edges per color (2 cores each)
EB = 512                        # edges per matmul block
LOG2 = float(np.log(2.0))

_F32 = mybir.dt.float32
_F32R = mybir.dt.float32r


def build_bass(chunks=None):
    chunks = CHUNKS if chunks is None else chunks
    e_core = sum(chunks)
    nc = _Bacc()
    x = nc.dram_tensor("x", [128, e_core // 2], _F32R, kind="ExternalInput")
    w1a_d = nc.dram_tensor("w1a", [128, F], _F32R, kind="ExternalInput")
    w1b_d = nc.dram_tensor("w1b", [128, F], _F32R, kind="ExternalInput")
    w2_d = nc.dram_tensor("w2", [F, F], _F32R, kind="ExternalInput")
    b1_d = nc.dram_tensor("b1", [F, 1], _F32, kind="ExternalInput")
    b2e_d = nc.dram_tensor("b2e", [F, 1], _F32, kind="ExternalInput")
    y = nc.dram_tensor("y", [F, e_core], _F32, kind="ExternalOutput")

    act_exp = mybir.ActivationFunctionType.Exp
    act_ln = mybir.ActivationFunctionType.Ln
    max_cols = max(chunks) // 2

    with TileContext(nc) as tc:
        with (
            tc.tile_pool(name="consts", bufs=1) as consts,
            tc.tile_pool(name="xp", bufs=4) as xp,
            tc.tile_pool(name="upool", bufs=2) as upool,
            tc.tile_pool(name="spool", bufs=2) as spool,
            tc.tile_pool(name="ypool", bufs=3) as ypool,
            tc.tile_pool(name="ph", bufs=1, space="PSUM") as ph,
            tc.tile_pool(name="py", bufs=4, space="PSUM") as py,
        ):
            # consts ride the Activation HWDGE queues: SP's dispatch stays
            # free for the first x chunk, and these queues are idle at
            # kernel start (outputs come much later), so the weights land
            # immediately instead of starving behind the x-prefetch flood.
            w1a_sb = consts.tile([128, F], _F32R)
            nc.scalar.dma_start(out=w1a_sb[:], in_=w1a_d[:, :])
            w1b_sb = consts.tile([128, F], _F32R)
            nc.scalar.dma_start(out=w1b_sb[:], in_=w1b_d[:, :])
            w2_sb = consts.tile([F, F], _F32R)
            nc.scalar.dma_start(out=w2_sb[:], in_=w2_d[:, :])
            b1_sb = consts.tile([F, 1], _F32)
            nc.scalar.dma_start(out=b1_sb[:], in_=b1_d[:, :])
            b2e_sb = consts.tile([F, 1], _F32)
            nc.scalar.dma_start(out=b2e_sb[:], in_=b2e_d[:, :])

            # Dummy activation: becomes the stream's first InstActivation so
            # Bacc's ACT_TABLE_LOAD lands here, in the preamble shadow,
            # instead of serializing behind the first real EXP's operands
            # (~10us saved).
            dm_in = consts.tile([1, 1], _F32)
            nc.vector.memset(dm_in[:], 0.0)
            dm_out = consts.tile([1, 1], _F32)
            nc.scalar.activation(dm_out[:], dm_in[:], act_exp, bias=0.0, scale=1.0)

            xoff = 0  # column offset into x (= edge offset / 2)
            yoff = 0  # column offset into y (= edge offset)
            for sz in chunks:
                cols = sz // 2
                nblk = cols // EB
                x_sb = xp.tile([128, max_cols], _F32R, tag="x")
                nc.sync.dma_start(
                    out=x_sb[:, :cols], in_=x[:, xoff : xoff + cols]
                )
                y_sb = ypool.tile([128, 2 * max_cols], _F32, tag="y")
                for t in range(2):  # t=0 -> edges on partitions 0-63
                    w1_sb = w1a_sb if t == 0 else w1b_sb
                    h_ps = ph.tile([F, max_cols], _F32, tag="h")
                    for j in range(nblk):
                        nc.tensor.matmul(
                            h_ps[:, j * EB : (j + 1) * EB],
                            lhsT=w1_sb[:],
                            rhs=x_sb[:, j * EB : (j + 1) * EB],
                            start=True,
                            stop=True,
                        )
                    u_sb = upool.tile([F, max_cols], _F32, tag="u")
                    nc.scalar.activation(
                        u_sb[:, :cols],
                        h_ps[:, :cols],
                        act_exp,
                        bias=b1_sb[:, 0:1],
                        scale=1.0,
                    )
                    s_sb = spool.tile([F, max_cols], _F32R, tag="s")
                    nc.scalar.activation(
                        s_sb[:, :cols], u_sb[:, :cols], act_ln, bias=1.0, scale=1.0
                    )
                    for j in range(nblk):
                        y_ps = py.tile([F, EB], _F32, tag="yp")
                        nc.tensor.matmul(
                            y_ps[:],
                            lhsT=w2_sb[:],
                            rhs=s_sb[:, j * EB : (j + 1) * EB],
                            start=True,
                            stop=True,
                        )
                        nc.vector.tensor_scalar_add(
                            y_sb[:, t * cols + j * EB : t * cols + (j + 1) * EB],
                            y_ps[:],
                            b2e_sb[:, 0:1],
                        )
                # out-DMAs ride the Activation-engine HWDGE queues so the
                # output bursts don't head-of-line-block SP's input
                # prefetch dispatches.  The final chunk streams out per
                # half-chunk so the kernel tail isn't one long DMA.
                if sz == chunks[-1] and yoff + sz == e_core:
                    nc.scalar.dma_start(
                        out=y[:, yoff : yoff + cols], in_=y_sb[:, :cols]
                    )
                    nc.scalar.dma_start(
                        out=y[:, yoff + cols : yoff + sz], in_=y_sb[:, cols:sz]
                    )
                else:
                    nc.scalar.dma_start(
                        out=y[:, yoff : yoff + sz], in_=y_sb[:, :sz]
                    )
                xoff += cols
                yoff += sz
    nc.finalize()
    return nc


def _prep_core_x(rows, chunks=None):
    """rows: [e_core, G] float32 -> interleaved [128, e_core//2]."""
    chunks = CHUNKS if chunks is None else chunks
    e_core = rows.shape[0]
    assert e_core == sum(chunks)
    out = np.empty((128, e_core // 2), np.float32)
    pos = 0
    col = 0
    for sz in chunks:
        half = sz // 2
        blk = rows[pos : pos + sz].reshape(2, half, G)  # t, e, g
        out[:64, col : col + half] = blk[0].T
        out[64:, col : col + half] = blk[1].T
        pos += sz
        col += half
    return out


def _core_in_map(rows, W1c, b1c, W2c, b2c, chunks=None):
    w1a = np.zeros((128, F), np.float32)
    w1a[:G] = W1c
    w1b = np.zeros((128, F), np.float32)
    w1b[G:] = W1c
    b2e = (b2c - LOG2 * W2c.sum(axis=0)).astype(np.float32).reshape(F, 1)
    return {
        "x": _prep_core_x(rows, chunks),
        "w1a": w1a,
        "w1b": w1b,
        "w2": np.ascontiguousarray(W2c, dtype=np.float32),
        "b1": np.asarray(b1c, np.float32).reshape(F, 1),
        "b2e": b2e,
    }


def _run(in_maps, nc=None, **kwargs):
    if nc is None:
        nc = build_bass()
    return run_bass_kernel_spmd(nc, in_maps, core_ids=list(range(N_CORES)), **kwargs)


def kernel(edge_attr, colors, W1, b1, W2, b2, _trace=False):
    edge_attr = np.ascontiguousarray(np.asarray(edge_attr, dtype=np.float32))
    colors_i = np.asarray(colors).astype(np.int64)
    W1 = np.asarray(W1, dtype=np.float32)
    b1 = np.asarray(b1, dtype=np.float32)
    W2 = np.asarray(W2, dtype=np.float32)
    b2 = np.asarray(b2, dtype=np.float32)

    idx = [np.flatnonzero(colors_i == c) for c in range(C)]
    if any(len(ix) > SEG for ix in idx):
        # Pathological color skew that the fixed 2-cores-per-color layout
        # cannot hold (impossible for the spec'd uniform randint fill).
        h = np.einsum("eg,cgf->cef", edge_attr, W1) + b1[:, None, :]
        h = np.logaddexp(h, 0.0) - LOG2
        yy = np.einsum("cef,cfh->ceh", h, W2) + b2[:, None, :]
        return np.ascontiguousarray(
            yy[colors_i, np.arange(edge_attr.shape[0])]
        ).astype(np.float32)

    in_maps = []
    for core in range(N_CORES):
        c = core // 2
        ix = idx[c]
        lo = (core % 2) * E_CORE
        rows = np.zeros((E_CORE, G), np.float32)
        take = ix[lo : lo + E_CORE]
        rows[: len(take)] = edge_attr[take]
        in_maps.append(_core_in_map(rows, W1[c], b1[c], W2[c], b2[c]))

    res = _run(in_maps, trace=_trace)

    out = np.empty((edge_attr.shape[0], F), np.float32)
    for c in range(C):
        ix = idx[c]
        n0 = min(len(ix), E_CORE)
        out[ix[:n0]] = res.results[2 * c]["y"][:, :n0].T
        if len(ix) > E_CORE:
            out[ix[E_CORE:]] = res.results[2 * c + 1]["y"][:, : len(ix) - E_CORE].T
    kernel.last_result = res
    return out


kernel.last_result = None


# revision 14
# speedup vs baseline: 1.0758x; 1.0758x over previous
"""Trainium2 Bass kernel for ColoredMLP (4-expert MoE over 500k edges).

Strategy (expert-parallel over colors, 2 cores per color):
  - Host groups edges by color (stable), pads each color segment to 126976,
    and assigns 2 cores per color.  Every core then runs an IDENTICAL dense
    single-expert MLP on 63488 edges with its own weight slice bound as
    inputs, so the device kernel is completely regular: no routing on
    device, no wasted all-expert compute.
  - Data layout: host ships x pre-transposed as [g, e] and interleaved so
    each DMA spans all 128 SBUF partitions: within a chunk of size S,
    x_in[t*64+g, e] = xT[g, t*(S/2) + e].  L1 uses zero-padded
    block-diagonal W1 slices (w1a = [W1;0], w1b = [0;W1]) so the matmul
    rhs is always a full-partition tile at base partition 0.
  - shifted_softplus(x) = softplus(x) - log2 is folded into the second
    layer's bias on host: b2_eff = b2 - log2 * W2.sum(0).  The device
    computes softplus as Ln(Exp(h + b1) + 1) — two ACT passes from the
    same activation-table set (this act_info has no native Softplus),
    batched wide to amortize ACT per-instruction overhead.  b1 rides the
    Exp pass as a per-partition ACT bias; b2_eff is a per-partition DVE
    scalar add fused into the PSUM->SBUF copy of y.
  - Matmuls run as float32r (full PE rate at N=512); PSUM accumulates fp32.
  - Input DMAs ride SP HWDGE queues, output DMAs ride the Activation
    HWDGE queues so output bursts never head-of-line-block input
    prefetch dispatch.

The kernel returns out^T tiles [128, 63488] per core; the host scatters
them back through the color permutation.
"""

import os
import sys

import numpy as np

if "/opt/trn_rl_repo" not in sys.path:
    sys.path.insert(0, "/opt/trn_rl_repo")

import bass_rust as _bass_rust
import concourse.bacc as bacc
import concourse.mybir as mybir
from concourse.hw_specs import get_activation_tables
from concourse.tile import TileContext
from concourse.bass_utils import run_bass_kernel_spmd


class _Bacc(bacc.Bacc):
    """Bacc that pins activation-table selection to the single set holding
    both Exp and Ln.  The default per-function choice alternates between
    `exp_and_others` and `natural_log`, inserting a ~1.3us ACT_TABLE_LOAD
    before every activation (82us of pure table thrash per core here)."""

    def insert_act_table_loads(self):
        has_activation = any(
            isinstance(i, mybir.InstActivation)
            for b in self.main_func.blocks
            for i in b.instructions
        )
        if not has_activation:
            return
        both = {
            mybir.ActivationFunctionType.Exp,
            mybir.ActivationFunctionType.Ln,
        }
        tables = []
        seen = False
        for k, fns in get_activation_tables(self.m.arch).items():
            if k == "natural_log_exp_and_others":
                seen = True
                assert both <= set(fns)
            else:
                fns = set(fns) - both
            tables.append((k, fns))
        assert seen, "natural_log_exp_and_others table set missing"
        _bass_rust.insert_act_table_loads(self, tables)


E, G, F, C = 500000, 64, 128, 4
N_CORES = 8
CHUNKS = [4096] * 15 + [2048]   # edges per DMA chunk
E_CORE = sum(CHUNKS)            # 63488 edges per core (fixed compile shape)
SEG = 2 * E_CORE                # 126976 padded edges per color (2 cores each)
EB = 512                        # edges per matmul block
LOG2 = float(np.log(2.0))

_F32 = mybir.dt.float32
_F32R = mybir.dt.float32r


def build_bass(chunks=None):
    chunks = CHUNKS if chunks is None else chunks
    e_core = sum(chunks)
    nc = _Bacc()
    x = nc.dram_tensor("x", [128, e_core // 2], _F32R, kind="ExternalInput")
    # all constants packed in one tensor: cols 0-127 w1a, 128-255 w1b,
    # 256-383 w2, 384 b1, 385 b2_eff
    cst_d = nc.dram_tensor("cst", [128, 3 * F + 2], _F32R, kind="ExternalInput")
    y = nc.dram_tensor("y", [F, e_core], _F32, kind="ExternalOutput")

    act_exp = mybir.ActivationFunctionType.Exp
    act_ln = mybir.ActivationFunctionType.Ln
    max_cols = max(chunks) // 2

    with TileContext(nc) as tc:
        with (
            tc.tile_pool(name="consts", bufs=1) as consts,
            tc.tile_pool(name="xp", bufs=4) as xp,
            tc.tile_pool(name="upool", bufs=2) as upool,
            tc.tile_pool(name="spool", bufs=2) as spool,
            tc.tile_pool(name="ypool", bufs=3) as ypool,
            tc.tile_pool(name="ph", bufs=1, space="PSUM") as ph,
            tc.tile_pool(name="py", bufs=4, space="PSUM") as py,
        ):
            # One packed const DMA, emitted first on SP HWDGE: it is the
            # first tick on its queue sem, so downstream const waits clear
            # immediately instead of entangling with x-chunk queue ticks.
            cst_sb = consts.tile([128, 3 * F + 2], _F32R)
            nc.sync.dma_start(out=cst_sb[:], in_=cst_d[:, :])
            w1a_sb = cst_sb[:, 0:F]
            w1b_sb = cst_sb[:, F : 2 * F]
            w2_sb = cst_sb[:, 2 * F : 3 * F]
            b1_sb = cst_sb[:, 3 * F : 3 * F + 1].bitcast(_F32)
            b2e_sb = cst_sb[:, 3 * F + 1 : 3 * F + 2].bitcast(_F32)

            # Dummy activation: becomes the stream's first InstActivation so
            # Bacc's ACT_TABLE_LOAD lands here, in the preamble shadow,
            # instead of serializing behind the first real EXP's operands
            # (~10us saved).
            dm_in = consts.tile([1, 1], _F32)
            nc.vector.memset(dm_in[:], 0.0)
            dm_out = consts.tile([1, 1], _F32)
            nc.scalar.activation(dm_out[:], dm_in[:], act_exp, bias=0.0, scale=1.0)

            xoff = 0  # column offset into x (= edge offset / 2)
            yoff = 0  # column offset into y (= edge offset)
            for sz in chunks:
                cols = sz // 2
                nblk = cols // EB
                x_sb = xp.tile([128, max_cols], _F32R, tag="x")
                nc.sync.dma_start(
                    out=x_sb[:, :cols], in_=x[:, xoff : xoff + cols]
                )
                y_sb = ypool.tile([128, 2 * max_cols], _F32, tag="y")
                for t in range(2):  # t=0 -> edges on partitions 0-63
                    w1_sb = w1a_sb if t == 0 else w1b_sb
                    h_ps = ph.tile([F, max_cols], _F32, tag="h")
                    for j in range(nblk):
                        nc.tensor.matmul(
                            h_ps[:, j * EB : (j + 1) * EB],
                            lhsT=w1_sb,
                            rhs=x_sb[:, j * EB : (j + 1) * EB],
                            start=True,
                            stop=True,
                        )
                    u_sb = upool.tile([F, max_cols], _F32, tag="u")
                    nc.scalar.activation(
                        u_sb[:, :cols],
                        h_ps[:, :cols],
                        act_exp,
                        bias=b1_sb,
                        scale=1.0,
                    )
                    s_sb = spool.tile([F, max_cols], _F32R, tag="s")
                    nc.scalar.activation(
                        s_sb[:, :cols], u_sb[:, :cols], act_ln, bias=1.0, scale=1.0
                    )
                    for j in range(nblk):
                        y_ps = py.tile([F, EB], _F32, tag="yp")
                        nc.tensor.matmul(
                            y_ps[:],
                            lhsT=w2_sb,
                            rhs=s_sb[:, j * EB : (j + 1) * EB],
                            start=True,
                            stop=True,
                        )
                        nc.vector.tensor_scalar_add(
                            y_sb[:, t * cols + j * EB : t * cols + (j + 1) * EB],
                            y_ps[:],
                            b2e_sb,
                        )
                # out-DMAs ride the Activation-engine HWDGE queues so the
                # output bursts don't head-of-line-block SP's input
                # prefetch dispatches.  The final chunk streams out per
                # half-chunk so the kernel tail isn't one long DMA.
                if sz == chunks[-1] and yoff + sz == e_core:
                    nc.scalar.dma_start(
                        out=y[:, yoff : yoff + cols], in_=y_sb[:, :cols]
                    )
                    nc.scalar.dma_start(
                        out=y[:, yoff + cols : yoff + sz], in_=y_sb[:, cols:sz]
                    )
                else:
                    nc.scalar.dma_start(
                        out=y[:, yoff : yoff + sz], in_=y_sb[:, :sz]
                    )
                xoff += cols
                yoff += sz
    nc.finalize()
    return nc


def _prep_core_x(rows, chunks=None):
    """rows: [e_core, G] float32 -> interleaved [128, e_core//2]."""
    chunks = CHUNKS if chunks is None else chunks
    e_core = rows.shape[0]
    assert e_core == sum(chunks)
    out = np.empty((128, e_core // 2), np.float32)
    pos = 0
    col = 0
    for sz in chunks:
        half = sz // 2
        blk = rows[pos : pos + sz].reshape(2, half, G)  # t, e, g
        out[:64, col : col + half] = blk[0].T
        out[64:, col : col + half] = blk[1].T
        pos += sz
        col += half
    return out


def _core_in_map(rows, W1c, b1c, W2c, b2c, chunks=None):
    cst = np.zeros((128, 3 * F + 2), np.float32)
    cst[:G, 0:F] = W1c
    cst[G:, F : 2 * F] = W1c
    cst[:, 2 * F : 3 * F] = W2c
    cst[:, 3 * F] = np.asarray(b1c, np.float32)
    cst[:, 3 * F + 1] = (b2c - LOG2 * W2c.sum(axis=0)).astype(np.float32)
    return {"x": _prep_core_x(rows, chunks), "cst": cst}


def _run(in_maps, nc=None, **kwargs):
    if nc is None:
        nc = build_bass()
    return run_bass_kernel_spmd(nc, in_maps, core_ids=list(range(N_CORES)), **kwargs)


def kernel(edge_attr, colors, W1, b1, W2, b2, _trace=False):
    edge_attr = np.ascontiguousarray(np.asarray(edge_attr, dtype=np.float32))
    colors_i = np.asarray(colors).astype(np.int64)
    W1 = np.asarray(W1, dtype=np.float32)
    b1 = np.asarray(b1, dtype=np.float32)
    W2 = np.asarray(W2, dtype=np.float32)
    b2 = np.asarray(b2, dtype=np.float32)

    idx = [np.flatnonzero(colors_i == c) for c in range(C)]
    if any(len(ix) > SEG for ix in idx):
        # Pathological color skew that the fixed 2-cores-per-color layout
        # cannot hold (impossible for the spec'd uniform randint fill).
        h = np.einsum("eg,cgf->cef", edge_attr, W1) + b1[:, None, :]
        h = np.logaddexp(h, 0.0) - LOG2
        yy = np.einsum("cef,cfh->ceh", h, W2) + b2[:, None, :]
        return np.ascontiguousarray(
            yy[colors_i, np.arange(edge_attr.shape[0])]
        ).astype(np.float32)

    in_maps = []
    for core in range(N_CORES):
        c = core // 2
        ix = idx[c]
        lo = (core % 2) * E_CORE
        rows = np.zeros((E_CORE, G), np.float32)
        take = ix[lo : lo + E_CORE]
        rows[: len(take)] = edge_attr[take]
        in_maps.append(_core_in_map(rows, W1[c], b1[c], W2[c], b2[c]))

    res = _run(in_maps, trace=_trace)

    out = np.empty((edge_attr.shape[0], F), np.float32)
    for c in range(C):
        ix = idx[c]
        n0 = min(len(ix), E_CORE)
        out[ix[:n0]] = res.results[2 * c]["y"][:, :n0].T
        if len(ix) > E_CORE:
            out[ix[E_CORE:]] = res.results[2 * c + 1]["y"][:, : len(ix) - E_CORE].T
    kernel.last_result = res
    return out


kernel.last_result = None
